# revision 54
# baseline (speedup 1.0000x reference)
"""BatchedTreeForest (moe_routing) Trainium2 kernel.

Reference computation (B=4, S=2048, D=1024, O=512, T=16 trees, depth 4):
  logits  = einsum('bsd,tnd->bstn', x, W_dec) + b_dec          (15 internal nodes)
  dec     = sigmoid(logits / softplus(temp_logits + .5413))
  leafp   = prod over the 4 root->leaf path levels of (dec | 1-dec)
  per_tree= einsum('bstl,tlo->bsto', leafp, leaf_outputs)
  gate    = softmax(x @ gate_w + gate_b)
  out     = LayerNorm(einsum('bsto,bst->bso', per_tree, gate)) * gamma + beta

Mapping onto 8 NeuronCores: data-parallel over the 8192 tokens (1024/core),
tree/gate parameters replicated.  fp16 on-device, fp32 PSUM accumulate.

Profile-driven design (vs the 47.5us stage_a/stage_b baseline):
  - All input DMAs issued up front in consumption order, spread over the
    three per-core DMA queues (sync HWDGE, scalar HWDGE, gpsimd SWDGE);
    the input stream is HBM-bound (~2.9MB at ~300GB/s), so mm1 is
    arrival-paced and everything else hides under it.
  - The first matmul's inputs (wcat k0:2 + xt0 k0:2) ride ONE merged
    "head" DMA: per-DMA completion is tile-granular, so splitting tiles
    by need avoids waiting on whole-tensor loads.
  - Single sigmoid pass over all 256 columns: the (1-s) complements are
    never materialized.  The cascade computes the om-halves as
    H*(s-1) = H*s - H (plain tensor_sub) and the sign is folded into
    host-permuted leaf_outputs rows ((-1)^popcount trick).
  - Bit-plane cascade (level-major, node-major columns via a host
    permutation of decision weights): every op is a contiguous
    unit-stride tensor op - no stride-0 broadcasts.  H1/H2 on DVE,
    H3/H4 on GpSimd.
  - Gate e = s/(1-s) from an extra 16-column sigmoid(-g) ACT pass
    (same ACT table), reciprocal on DVE; Z folded into the LN epsilon:
    rstd = 1/sqrt(var + eps*Z^2) via magic seed + one Newton step.
  - Three-stage software pipeline with split emission: transposes(p)
    go before mm1(q=p+3) so the ACT psum->sbuf copy hides under the mm1
    block and mm2(p) lands right after it with no PE bubble; the gpsimd
    cascade of supertile p gets two whole mm1 blocks of slack.  The
    stage_a DVE ops (gate+H1/H2) are emitted after mm2_pair's stats so
    the in-order DVE queue isn't head-of-line blocked.
  - LN finalize per tile-pair right after its mm2 frees PSUM banks (no
    PE stall on psum pool exhaustion); the last pair finalizes per-tile,
    tile 7 first with its out DMA split across both HWDGE queues.
  - Engine balance: ACT = sigmoids + psum->sbuf copies + LN applies;
    DVE = gate + cascade H1/H2 + bn_stats/aggr + LN chain;
    GpSimd = cascade H3/H4 + SWDGE loads.
"""
import sys

sys.path.insert(0, "/opt/trn_rl_repo")

import numpy as np

P = 128
D = 1024
T = 16
NI = 15
NL = 16
NDEC = T * NI  # 240
COLS = NDEC + T  # 256: decision logits | gate logits
O = 512
NCORES = 8
TOK_PC = 1024  # tokens per core
NTILES = TOK_PC // P  # 8
NST = NTILES // 2  # 4 supertiles of 2 tiles
KT = D // P  # 8 contraction tiles
EPS = 1e-5
MAGIC = 0x5F3759DF


def _bitrev(x, bits):
    r = 0
    for _ in range(bits):
        r = (r << 1) | (x & 1)
        x >>= 1
    return r


def _dec_order():
    """Column order of the 240 decision columns: level-major blocks, within a
    block node-major (c = LSB-first path prefix), tree-inner."""
    order = []
    for d in range(4):
        for c in range(2**d):
            node = (2**d - 1) + _bitrev(c, d)
            for t in range(T):
                order.append(t * NI + node)
    return np.array(order)


def _leaf_perm_signs():
    """leafp2 row g = b4*128 + v3*16 + t holds sign * leaf_outputs[t, l_std],
    sign = (-1)^(b1+b2+b3+b4) compensating the (s-1) cascade ops."""
    perm = np.zeros(256, np.int64)
    sign = np.zeros(256, np.float32)
    for b4 in range(2):
        for v3 in range(8):
            b1, b2, b3 = v3 & 1, (v3 >> 1) & 1, (v3 >> 2) & 1
            for t in range(T):
                g = b4 * 128 + v3 * 16 + t
                perm[g] = t * NL + (b1 * 8 + b2 * 4 + b3 * 2 + b4)
                # extra global -1 compensates the gate ge = s/(s-1) = -e
                sign[g] = 1.0 if ((b1 + b2 + b3 + b4) % 2) else -1.0
    return perm, sign


def build(apply_affine: bool = False, has_bias: bool = False):
    """Build the per-core Bass module.  Returns the Bacc object (uncompiled)."""
    import concourse.bacc as bacc
    import concourse.mybir as mybir
    from concourse import masks
    from concourse.tile import TileContext

    f32 = mybir.dt.float32
    f16 = mybir.dt.float16
    i32 = mybir.dt.int32
    Alu = mybir.AluOpType
    Act = mybir.ActivationFunctionType
    X = mybir.AxisListType.X

    nc = bacc.Bacc()
    # Host-prepped layouts (see _host_prep):
    #   xp[j*P+p, k*P+t]   = x[core, j*P+t, k*P+p]   (tile-major, 2KB lines)
    #   wcatp[p, k*COLS+c] = wcat[c, k*P+p]          (c in _dec_order | gate)
    #   leafp2[p, c*O+o]   = signed/permuted leaf rows (see _leaf_perm_signs)
    xp_d = nc.declare_dram_parameter("xp", [NTILES * P, KT * P], f16, isOutput=False)
    wcat_d = nc.declare_dram_parameter("wcatp", [P, KT * COLS], f16, isOutput=False)
    # headp = [wcat k0:2 | xt0 k0:2] merged so ONE first DMA unblocks mm1
    head_d = nc.declare_dram_parameter(
        "headp", [P, 2 * COLS + 2 * P], f16, isOutput=False
    )
    if has_bias:
        bias_d = nc.declare_dram_parameter("biascat", [2, COLS], f16, isOutput=False)
    leaf_d = nc.declare_dram_parameter("leafp2", [P, 2 * O], f16, isOutput=False)
    if apply_affine:
        gamma_d = nc.declare_dram_parameter("gamma", [1, O], f32, isOutput=False)
        beta_d = nc.declare_dram_parameter("beta", [1, O], f32, isOutput=False)
    out_d = nc.declare_dram_parameter("out", [TOK_PC, O], f16, isOutput=True)

    with TileContext(nc) as tc:
        with (
            tc.tile_pool(name="consts", bufs=1) as consts,
            tc.tile_pool(name="xin", bufs=NTILES) as xin,
            tc.tile_pool(name="d2p", bufs=4) as d2p,
            tc.tile_pool(name="casc", bufs=4) as casc,
            tc.tile_pool(name="accp", bufs=4) as accp,
            tc.tile_pool(name="wtp", bufs=3) as wtp,
            tc.tile_pool(name="outp", bufs=4) as outp,
            tc.tile_pool(name="gatep", bufs=4) as gatep,
            tc.tile_pool(name="smalls", bufs=4) as smalls,
            tc.tile_pool(name="psum1", bufs=2, space="PSUM") as psum1,
            tc.tile_pool(name="psumT", bufs=2, space="PSUM") as psumT,
            tc.tile_pool(name="psum3", bufs=4, space="PSUM") as psum3,
        ):
            # ---- startup loads: everything issued up front, spread across
            # the sync + scalar HWDGE queues and the gpsimd SWDGE queue, in
            # the order mm1 consumes it.  Tile-granular DMA dependencies:
            # wcat and xt0 are split into separate tiles so the first
            # matmuls wait only on the first small chunks.
            wcat_r = wcat_d[:, :].rearrange("p (k c) -> p k c", k=KT)
            head_sb = consts.tile([P, 2 * COLS + 2 * P], f16)  # wcat k0:2 | xt0 k0:2
            wcat_mid = consts.tile([P, 3, COLS], f16)  # k 2:5
            wcat_hi = consts.tile([P, 3, COLS], f16)   # k 5:8

            def wsel(k):
                if k < 2:
                    return head_sb[:, k * COLS : (k + 1) * COLS]
                if k < 5:
                    return wcat_mid[:, k - 2]
                return wcat_hi[:, k - 5]

            xts = [
                xin.tile([P, KT, P], f16, name="xt") if j else None
                for j in range(NTILES)
            ]
            xt0_hi = xin.tile([P, 6, P], f16, name="xt0hi")

            def xsel(j, k):
                if j == 0:
                    if k < 2:
                        off = 2 * COLS + k * P
                        return head_sb[:, off : off + P]
                    return xt0_hi[:, k - 2]
                return xts[j][:, k]

            def xsrc(j):
                return xp_d[j * P : (j + 1) * P, :].rearrange(
                    "p (k t) -> p k t", k=KT
                )

            # q1 carries the bulk in need-order; the merged head rides q10
            # (which starts ~0.5us later) so mm1's k0 unblocks at ~10.3 with
            # the rest of supertile 0 already close behind - the PE then
            # runs gap-free instead of starting early and de-ramping.
            nc.sync.dma_start(out=wcat_mid, in_=wcat_r[:, 2:5])
            nc.sync.dma_start(out=xt0_hi, in_=xsrc(0)[:, 2:8])
            nc.sync.dma_start(out=xts[1], in_=xsrc(1))
            nc.sync.dma_start(out=xts[2], in_=xsrc(2))
            nc.sync.dma_start(out=xts[4], in_=xsrc(4))
            nc.sync.dma_start(out=xts[6], in_=xsrc(6))
            # scalar (q10): issues after the ACT table loads (~7us)
            nc.scalar.dma_start(out=head_sb, in_=head_d[:, :])
            nc.scalar.dma_start(out=wcat_hi, in_=wcat_r[:, 5:8])
            nc.scalar.dma_start(out=xts[5], in_=xsrc(5))
            if has_bias:
                bias_sb = consts.tile([1, COLS], f16)
                nc.scalar.dma_start(out=bias_sb, in_=bias_d[0:1, :])
                ones_sb = consts.tile([1, P], f16)
                nc.scalar.dma_start(out=ones_sb, in_=bias_d[1:2, 0:P])
            # gpsimd SWDGE (q0): late-needed tiles
            leaf_sb = consts.tile([P, 2, O], f16)
            nc.gpsimd.dma_start(
                out=leaf_sb, in_=leaf_d[:, :].rearrange("p (c o) -> p c o", c=2)
            )
            nc.gpsimd.dma_start(out=xts[3], in_=xsrc(3))
            nc.gpsimd.dma_start(out=xts[7], in_=xsrc(7))
            ident = consts.tile([P, P], f16)
            masks.make_identity(nc, ident[:, :])
            if apply_affine:
                gamma_sb = consts.tile([P, O], f32)
                nc.gpsimd.dma_start(
                    out=gamma_sb, in_=gamma_d[:, :].to_broadcast((P, O))
                )
                beta_sb = consts.tile([P, O], f32)
                nc.gpsimd.dma_start(out=beta_sb, in_=beta_d[:, :].to_broadcast((P, O)))

            state = {}
            gz = gatep.tile([P, NTILES], f32, name="gz", tag="gz")

            sa_ctx = {}

            def stage_a1(q):
                """mm1 + sigmoid + gate-complement for supertile q."""
                ps1 = psum1.tile([P, 2, COLS], f32, name="ps1")
                for a in range(2):
                    j = 2 * q + a
                    for k in range(KT):
                        nc.tensor.matmul(
                            ps1[:, a], xsel(j, k), wsel(k), start=(k == 0),
                            stop=(not has_bias and k == KT - 1),
                        )
                    if has_bias:
                        nc.tensor.matmul(
                            ps1[:, a], ones_sb, bias_sb, start=False, stop=True
                        )

                # ---- single sigmoid pass over all 256 columns; the gate
                # complement is derived from s on DVE (s-1 is exact in fp16
                # for s>=0.5 by Sterbenz), so no second ACT pass exists to
                # queue behind copies/outs on the scalar engine ----
                d2 = d2p.tile([P, 2, COLS], f16, name="d2")
                nc.scalar.activation(out=d2, in_=ps1, func=Act.Sigmoid)
                sa_ctx[q] = d2

            def stage_a2(q):
                """gate e = s/(1-s) + bit-plane cascade for supertile q;
                emitted after mm2_pair so the DVE stats/chain of the older
                pair aren't head-of-line blocked behind these later-ready
                ops."""
                d2 = sa_ctx.pop(q)
                # ge = s * 1/(s-1) = -e; the global -1 is folded into the
                # leafp2 row signs on the host (gz = -Z squares away)
                sm1 = gatep.tile([P, 2, T], f16, name="som")
                nc.vector.tensor_scalar(
                    out=sm1, in0=d2[:, :, NDEC:COLS], scalar1=1.0, scalar2=None,
                    op0=Alu.subtract,
                )
                gr = gatep.tile([P, 2, T], f32, name="gr")
                nc.vector.reciprocal(gr, sm1)
                ge = gatep.tile([P, 2, T], f16, name="ge")
                nc.vector.scalar_tensor_tensor(
                    out=ge, in0=d2[:, :, NDEC:COLS], scalar=1.0, in1=gr,
                    op0=Alu.mult, op1=Alu.mult,
                )
                nc.vector.reduce_sum(gz[:, 2 * q : 2 * q + 2], ge, axis=X)

                # ---- bit-plane cascade, all unit-stride plain TT ops ----
                # level-(d+1) block cols: [c*T + t], node 2^d-1+bitrev(c).
                # H_d index v_d (LSB-first path prefix), tree-inner.
                # om-halves via H*(s-1) = H*s - H (tensor_sub of the s-half);
                # the sign is folded into leafp2 rows.  H1/H2 on DVE (tiny),
                # H3/H4 on GpSimd.
                roots = d2[:, :, 0:T]
                l2 = d2[:, :, T : 3 * T].rearrange("p a (c t) -> p a c t", c=2)
                l3 = d2[:, :, 3 * T : 7 * T].rearrange("p a (c t) -> p a c t", c=4)
                l4 = d2[:, :, 7 * T : 15 * T].rearrange("p a (c t) -> p a c t", c=8)
                h1 = casc.tile([P, 2, 2, T], f16, name="h1")
                nc.vector.tensor_mul(h1[:, :, 0], ge, roots)
                nc.vector.tensor_sub(h1[:, :, 1], h1[:, :, 0], ge)
                h2 = casc.tile([P, 2, 4, T], f16, name="h2")
                nc.vector.tensor_mul(h2[:, :, 0:2], h1, l2)
                nc.vector.tensor_sub(h2[:, :, 2:4], h2[:, :, 0:2], h1)
                # supertile 3's cascade gates the final transposes+mm2: the
                # gpsimd queue retires it late, so run the last one on DVE
                # where the queue is clear right after H2(3)
                e34 = nc.vector if q == NST - 1 else nc.gpsimd
                h3 = casc.tile([P, 2, 8, T], f16, name="h3")
                e34.tensor_mul(h3[:, :, 0:4], h2, l3)
                e34.tensor_sub(h3[:, :, 4:8], h3[:, :, 0:4], h2)
                acc = accp.tile([P, 2, NL, T], f16, name="acc")
                e34.tensor_mul(acc[:, :, 0:8], h3, l4)
                e34.tensor_sub(acc[:, :, 8:16], acc[:, :, 0:8], h3)
                state[q] = acc

            def ln_finalize(jbase, w, mvw, pair, split_dma=False):
                """LN for tiles jbase..jbase+w-1: rstd via magic seed + one
                Newton step, apply on ACT, out DMA on sync/scalar."""
                fin_prio = tc.high_priority(offset=120)
                fin_prio.__enter__()
                gzb = gz[:, jbase : jbase + w]
                # vt = var + eps*Z^2
                vt = smalls.tile([P, 2], f32, name="vt")[:, 0:w]
                nc.vector.scalar_tensor_tensor(
                    out=vt, in0=gzb, scalar=float(EPS), in1=gzb,
                    op0=Alu.mult, op1=Alu.mult,
                )
                nc.vector.tensor_add(vt, vt, mvw[:, :, 1])
                iv = smalls.tile([P, 2], i32, name="iv")[:, 0:w]
                nc.vector.tensor_scalar(
                    out=iv, in0=vt.bitcast(i32), scalar1=1, scalar2=None,
                    op0=Alu.logical_shift_right,
                )
                yt = smalls.tile([P, 2], f32, name="yt")[:, 0:w]
                nc.vector.tensor_scalar(
                    out=yt.bitcast(i32), in0=iv, scalar1=-1, scalar2=MAGIC,
                    op0=Alu.mult, op1=Alu.add,
                )
                aq = smalls.tile([P, 2], f32, name="aq")[:, 0:w]
                nc.vector.tensor_mul(aq, yt, yt)
                bq = smalls.tile([P, 2], f32, name="bq")[:, 0:w]
                nc.vector.scalar_tensor_tensor(
                    out=bq, in0=vt, scalar=0.5, in1=aq, op0=Alu.mult, op1=Alu.mult
                )
                cq = smalls.tile([P, 2], f32, name="cq")[:, 0:w]
                nc.vector.tensor_scalar(
                    out=cq, in0=bq, scalar1=-1.0, scalar2=1.5,
                    op0=Alu.mult, op1=Alu.add,
                )
                nc.vector.tensor_mul(yt, yt, cq)
                nb = smalls.tile([P, 2], f32, name="nb")[:, 0:w]
                nc.vector.scalar_tensor_tensor(
                    out=nb, in0=mvw[:, :, 0], scalar=-1.0, in1=yt,
                    op0=Alu.mult, op1=Alu.mult,
                )
                for idx, (j, ps3) in enumerate(pair):
                    out_sb = outp.tile([P, O], f16, name="out_sb")
                    nc.scalar.activation(
                        out=out_sb, in_=ps3, func=Act.Identity,
                        bias=nb[:, idx : idx + 1], scale=yt[:, idx : idx + 1],
                    )
                    if apply_affine:
                        nc.vector.tensor_mul(out_sb, out_sb, gamma_sb)
                        nc.vector.tensor_add(out_sb, out_sb, beta_sb)
                    dst = out_d[j * P : (j + 1) * P, :]
                    if split_dma and j == NTILES - 1:
                        half = O // 2
                        nc.scalar.dma_start(out=dst[:, 0:half], in_=out_sb[:, 0:half])
                        nc.sync.dma_start(out=dst[:, half:O], in_=out_sb[:, half:O])
                    else:
                        eng = nc.scalar if j >= 6 else nc.sync
                        eng.dma_start(out=dst, in_=out_sb)
                fin_prio.__exit__(None, None, None)

            wts = {}

            def transposes(p):
                """PE transposes of acc(p) + the ACT psum->sbuf copy; the
                copy overlaps the mm1 block emitted right after."""
                acc = state.pop(p)
                accf = acc.rearrange("p a v t -> p (a v t)")
                psT = psumT.tile([P, 4, P], f16, name="psT")
                for c in range(4):
                    nc.tensor.transpose(psT[:, c], accf[:, c * P : (c + 1) * P], ident)
                wt = wtp.tile([P, 4, P], f16, name="wt")
                nc.scalar.copy(
                    wt.rearrange("p c t -> p (c t)"), psT.rearrange("p c t -> p (c t)")
                )
                wts[p] = wt

            def mm2_pair(p):
                wt = wts.pop(p)
                last = p == NST - 1
                mvp = gatep.tile([P, 2, 2], f32, name="mvp")
                pair = []
                # last pair: tile 7 first and finalized per-tile, so its out
                # DMA (split across both queues) starts as early as possible
                for a in (1, 0) if last else (0, 1):
                    j = 2 * p + a
                    ps3 = psum3.tile([P, O], f32, name="ps3")
                    nc.tensor.matmul(
                        ps3, wt[:, 2 * a], leaf_sb[:, 0], start=True, stop=False
                    )
                    nc.tensor.matmul(
                        ps3, wt[:, 2 * a + 1], leaf_sb[:, 1], start=False, stop=True
                    )
                    st6 = smalls.tile([P, 6], f32, name="st6")
                    nc.vector.bn_stats(st6, ps3)
                    nc.vector.bn_aggr(mvp[:, a], st6)
                    if last:
                        ln_finalize(
                            j, 1, mvp[:, a : a + 1], [(j, ps3)], split_dma=True
                        )
                    else:
                        pair.append((j, ps3))
                if not last:
                    ln_finalize(2 * p, 2, mvp, pair)

            # ---- software pipeline, three stages deep: transposes(p) go
            # before mm1(q) so the psum->sbuf copy hides under the mm1
            # block, and mm2(p) lands right after it; the gpsimd cascade of
            # supertile p gets two whole mm1 blocks of slack ----
            for it in range(NST + 3):
                q, p = it, it - 3
                if p >= 0:
                    transposes(p)
                if q < NST:
                    stage_a1(q)
                if p >= 0:
                    mm2_pair(p)
                if q < NST:
                    stage_a2(q)

    return nc


def _host_prep(x, decision_weights, decision_biases, leaf_outputs, gate_w, gate_b,
               node_temp_logits, ln_gamma, ln_beta):
    """Fold temperatures into weights/biases, permute columns for the
    bit-plane cascade, sign/permute leaf rows, cast fp16, shard tokens."""
    x = np.asarray(x, np.float32)
    temps = np.log1p(np.exp(np.asarray(node_temp_logits, np.float64) + 0.5413))
    temps = temps.astype(np.float32)  # TEMP == 1.0
    wd = (np.asarray(decision_weights, np.float32) / temps[..., None]).reshape(NDEC, D)
    order = _dec_order()
    wcat = np.concatenate(
        [wd[order], np.asarray(gate_w, np.float32).T], axis=0
    )  # [256, D]
    # wcatp[p, k*COLS + c] = wcat[c, k*P + p]
    wcatp = np.ascontiguousarray(
        wcat.T.reshape(KT, P, COLS).transpose(1, 0, 2).reshape(P, KT * COLS)
    ).astype(np.float16)
    biasrow = np.concatenate(
        [
            (np.asarray(decision_biases, np.float32) / temps).reshape(NDEC)[order],
            np.asarray(gate_b, np.float32),
        ]
    )
    biascat = np.stack([biasrow, np.ones(COLS, np.float32)]).astype(np.float16)
    # leafp2[p, c*O + o] = sign[c*P+p] * leaf_outputs[perm[c*P+p], o]
    perm, sign = _leaf_perm_signs()
    L2 = sign[:, None] * np.asarray(leaf_outputs, np.float32).reshape(T * NL, O)[perm]
    leafp2 = np.ascontiguousarray(
        L2.reshape(2, P, O).transpose(1, 0, 2).reshape(P, 2 * O)
    ).astype(np.float16)
    # xp[j*P + p, k*P + t] = xT[k*P + p, j*P + t] per core
    tokens = x.reshape(NCORES, TOK_PC, D)
    xps = []
    for c in range(NCORES):
        xT = tokens[c].T.astype(np.float16)  # [D, TOK_PC]
        xp = (
            xT.reshape(KT, P, NTILES, P)
            .transpose(2, 1, 0, 3)
            .reshape(NTILES * P, KT * P)
        )
        xps.append(np.ascontiguousarray(xp))
    gamma = np.asarray(ln_gamma, np.float32)
    beta = np.asarray(ln_beta, np.float32)
    affine = not (np.all(gamma == 1.0) and np.all(beta == 0.0))
    has_bias = bool(np.any(biasrow != 0.0))
    # merged head: wcat k0:2 cols | xt0 k0:2 per core
    heads = [
        np.ascontiguousarray(
            np.concatenate([wcatp[:, 0 : 2 * COLS], xp[0:P, 0 : 2 * P]], axis=1)
        )
        for xp in xps
    ]
    return xps, wcatp, biascat, leafp2, gamma, beta, affine, has_bias, heads


_BUILT = {}


def _get_module(apply_affine, has_bias):
    key = (apply_affine, has_bias)
    if key not in _BUILT:
        nc = build(apply_affine, has_bias)
        nc.compile()
        _BUILT[key] = nc
    return _BUILT[key]


def run_shards(in_maps, apply_affine=False, has_bias=False, trace=False):
    from concourse.bass_utils import run_bass_kernel_spmd

    nc = _get_module(apply_affine, has_bias)
    return run_bass_kernel_spmd(nc, in_maps, list(range(NCORES)), trace=trace)


def make_in_maps(inputs):
    (xps, wcatp, biascat, leafp2, gamma, beta, affine, has_bias, heads) = _host_prep(
        **inputs
    )
    in_maps = []
    for c in range(NCORES):
        m = {"xp": xps[c], "wcatp": wcatp, "leafp2": leafp2, "headp": heads[c]}
        if has_bias:
            m["biascat"] = biascat
        if affine:
            m["gamma"] = gamma[None, :]
            m["beta"] = beta[None, :]
        in_maps.append(m)
    return in_maps, affine, has_bias


def kernel(**inputs) -> np.ndarray:
    B, S = inputs["x"].shape[:2]
    in_maps, affine, has_bias = make_in_maps(inputs)
    res = run_shards(in_maps, apply_affine=affine, has_bias=has_bias)
    out = np.concatenate([res.results[c]["out"] for c in range(NCORES)], axis=0)
    return out.reshape(B, S, O).astype(np.float32)


# revision 57
# speedup vs baseline: 1.0508x; 1.0508x over previous
"""BatchedTreeForest (moe_routing) Trainium2 kernel.

Reference computation (B=4, S=2048, D=1024, O=512, T=16 trees, depth 4):
  logits  = einsum('bsd,tnd->bstn', x, W_dec) + b_dec          (15 internal nodes)
  dec     = sigmoid(logits / softplus(temp_logits + .5413))
  leafp   = prod over the 4 root->leaf path levels of (dec | 1-dec)
  per_tree= einsum('bstl,tlo->bsto', leafp, leaf_outputs)
  gate    = softmax(x @ gate_w + gate_b)
  out     = LayerNorm(einsum('bsto,bst->bso', per_tree, gate)) * gamma + beta

Mapping onto 8 NeuronCores: data-parallel over the 8192 tokens (1024/core),
tree/gate parameters replicated.  fp16 on-device, fp32 PSUM accumulate.

Profile-driven design (vs the 47.5us stage_a/stage_b baseline):
  - All input DMAs issued up front in consumption order, spread over the
    three per-core DMA queues (sync HWDGE, scalar HWDGE, gpsimd SWDGE);
    the input stream is HBM-bound (~2.9MB at ~300GB/s), so mm1 is
    arrival-paced and everything else hides under it.
  - The first matmul's inputs (wcat k0:2 + xt0 k0:2) ride ONE merged
    "head" DMA: per-DMA completion is tile-granular, so splitting tiles
    by need avoids waiting on whole-tensor loads.
  - Single sigmoid pass over all 256 columns: the (1-s) complements are
    never materialized.  The cascade computes the om-halves as
    H*(s-1) = H*s - H (plain tensor_sub) and the sign is folded into
    host-permuted leaf_outputs rows ((-1)^popcount trick).
  - Bit-plane cascade (level-major, node-major columns via a host
    permutation of decision weights): every op is a contiguous
    unit-stride tensor op - no stride-0 broadcasts.  H1/H2 on DVE,
    H3/H4 on GpSimd.
  - Gate e = s/(1-s) from an extra 16-column sigmoid(-g) ACT pass
    (same ACT table), reciprocal on DVE; Z folded into the LN epsilon:
    rstd = 1/sqrt(var + eps*Z^2) via magic seed + one Newton step.
  - Three-stage software pipeline with split emission: transposes(p)
    go before mm1(q=p+3) so the ACT psum->sbuf copy hides under the mm1
    block and mm2(p) lands right after it with no PE bubble; the gpsimd
    cascade of supertile p gets two whole mm1 blocks of slack.  The
    stage_a DVE ops (gate+H1/H2) are emitted after mm2_pair's stats so
    the in-order DVE queue isn't head-of-line blocked.
  - LN finalize per tile-pair right after its mm2 frees PSUM banks (no
    PE stall on psum pool exhaustion); the last pair finalizes per-tile,
    tile 7 first with its out DMA split across both HWDGE queues.
  - Engine balance: ACT = sigmoids + psum->sbuf copies + LN applies;
    DVE = gate + cascade H1/H2 + bn_stats/aggr + LN chain;
    GpSimd = cascade H3/H4 + SWDGE loads.
"""
import sys

sys.path.insert(0, "/opt/trn_rl_repo")

import numpy as np

P = 128
D = 1024
T = 16
NI = 15
NL = 16
NDEC = T * NI  # 240
COLS = NDEC + T  # 256: decision logits | gate logits
O = 512
NCORES = 8
TOK_PC = 1024  # tokens per core
NTILES = TOK_PC // P  # 8
NST = NTILES // 2  # 4 supertiles of 2 tiles
KT = D // P  # 8 contraction tiles
EPS = 1e-5
MAGIC = 0x5F3759DF


def _bitrev(x, bits):
    r = 0
    for _ in range(bits):
        r = (r << 1) | (x & 1)
        x >>= 1
    return r


def _dec_order():
    """Column order of the 240 decision columns: level-major blocks, within a
    block node-major (c = LSB-first path prefix), tree-inner."""
    order = []
    for d in range(4):
        for c in range(2**d):
            node = (2**d - 1) + _bitrev(c, d)
            for t in range(T):
                order.append(t * NI + node)
    return np.array(order)


def _leaf_perm_signs():
    """leafp2 row g = b4*128 + v3*16 + t holds sign * leaf_outputs[t, l_std],
    sign = (-1)^(b1+b2+b3+b4) compensating the (s-1) cascade ops."""
    perm = np.zeros(256, np.int64)
    sign = np.zeros(256, np.float32)
    for b4 in range(2):
        for v3 in range(8):
            b1, b2, b3 = v3 & 1, (v3 >> 1) & 1, (v3 >> 2) & 1
            for t in range(T):
                g = b4 * 128 + v3 * 16 + t
                perm[g] = t * NL + (b1 * 8 + b2 * 4 + b3 * 2 + b4)
                sign[g] = -1.0 if ((b1 + b2 + b3 + b4) % 2) else 1.0
    return perm, sign


def build(apply_affine: bool = False, has_bias: bool = False):
    """Build the per-core Bass module.  Returns the Bacc object (uncompiled)."""
    import concourse.bacc as bacc
    import concourse.mybir as mybir
    from concourse import masks
    from concourse.tile import TileContext

    f32 = mybir.dt.float32
    f16 = mybir.dt.float16
    i32 = mybir.dt.int32
    Alu = mybir.AluOpType
    Act = mybir.ActivationFunctionType
    X = mybir.AxisListType.X

    nc = bacc.Bacc()
    # Host-prepped layouts (see _host_prep):
    #   xp[j*P+p, k*P+t]   = x[core, j*P+t, k*P+p]   (tile-major, 2KB lines)
    #   wcatp[p, k*COLS+c] = wcat[c, k*P+p]          (c in _dec_order | gate)
    #   leafp2[p, c*O+o]   = signed/permuted leaf rows (see _leaf_perm_signs)
    xp_d = nc.declare_dram_parameter("xp", [NTILES * P, KT * P], f16, isOutput=False)
    wcat_d = nc.declare_dram_parameter("wcatp", [P, KT * COLS], f16, isOutput=False)
    # headp = [wcat k0:2 | xt0 k0:2] merged so ONE first DMA unblocks mm1
    head_d = nc.declare_dram_parameter(
        "headp", [P, 2 * COLS + 2 * P], f16, isOutput=False
    )
    if has_bias:
        bias_d = nc.declare_dram_parameter("biascat", [2, COLS], f16, isOutput=False)
    leaf_d = nc.declare_dram_parameter("leafp2", [P, 2 * O], f16, isOutput=False)
    if apply_affine:
        gamma_d = nc.declare_dram_parameter("gamma", [1, O], f32, isOutput=False)
        beta_d = nc.declare_dram_parameter("beta", [1, O], f32, isOutput=False)
    out_d = nc.declare_dram_parameter("out", [TOK_PC, O], f16, isOutput=True)

    with TileContext(nc) as tc:
        with (
            tc.tile_pool(name="consts", bufs=1) as consts,
            tc.tile_pool(name="xin", bufs=NTILES) as xin,
            tc.tile_pool(name="d2p", bufs=4) as d2p,
            tc.tile_pool(name="casc", bufs=4) as casc,
            tc.tile_pool(name="accp", bufs=4) as accp,
            tc.tile_pool(name="wtp", bufs=3) as wtp,
            tc.tile_pool(name="outp", bufs=4) as outp,
            tc.tile_pool(name="gatep", bufs=4) as gatep,
            tc.tile_pool(name="smalls", bufs=4) as smalls,
            tc.tile_pool(name="psum1", bufs=2, space="PSUM") as psum1,
            tc.tile_pool(name="psumT", bufs=2, space="PSUM") as psumT,
            tc.tile_pool(name="psum3", bufs=4, space="PSUM") as psum3,
        ):
            # ---- startup loads: everything issued up front, spread across
            # the sync + scalar HWDGE queues and the gpsimd SWDGE queue, in
            # the order mm1 consumes it.  Tile-granular DMA dependencies:
            # wcat and xt0 are split into separate tiles so the first
            # matmuls wait only on the first small chunks.
            wcat_r = wcat_d[:, :].rearrange("p (k c) -> p k c", k=KT)
            head_sb = consts.tile([P, 2 * COLS + 2 * P], f16)  # wcat k0:2 | xt0 k0:2
            wcat_mid = consts.tile([P, 3, COLS], f16)  # k 2:5
            wcat_hi = consts.tile([P, 3, COLS], f16)   # k 5:8

            def wsel(k):
                if k < 2:
                    return head_sb[:, k * COLS : (k + 1) * COLS]
                if k < 5:
                    return wcat_mid[:, k - 2]
                return wcat_hi[:, k - 5]

            xts = [
                xin.tile([P, KT, P], f16, name="xt") if j else None
                for j in range(NTILES)
            ]
            xt0_hi = xin.tile([P, 6, P], f16, name="xt0hi")

            def xsel(j, k):
                if j == 0:
                    if k < 2:
                        off = 2 * COLS + k * P
                        return head_sb[:, off : off + P]
                    return xt0_hi[:, k - 2]
                return xts[j][:, k]

            def xsrc(j):
                return xp_d[j * P : (j + 1) * P, :].rearrange(
                    "p (k t) -> p k t", k=KT
                )

            # q1 carries the bulk in need-order; the merged head rides q10
            # (which starts ~0.5us later) so mm1's k0 unblocks at ~10.3 with
            # the rest of supertile 0 already close behind - the PE then
            # runs gap-free instead of starting early and de-ramping.
            nc.sync.dma_start(out=wcat_mid, in_=wcat_r[:, 2:5])
            nc.sync.dma_start(out=xt0_hi, in_=xsrc(0)[:, 2:8])
            nc.sync.dma_start(out=xts[1], in_=xsrc(1))
            nc.sync.dma_start(out=xts[2], in_=xsrc(2))
            nc.sync.dma_start(out=xts[4], in_=xsrc(4))
            nc.sync.dma_start(out=xts[6], in_=xsrc(6))
            # scalar (q10): issues after the ACT table loads (~7us)
            nc.scalar.dma_start(out=head_sb, in_=head_d[:, :])
            nc.scalar.dma_start(out=wcat_hi, in_=wcat_r[:, 5:8])
            nc.scalar.dma_start(out=xts[5], in_=xsrc(5))
            if has_bias:
                bias_sb = consts.tile([1, COLS], f16)
                nc.scalar.dma_start(out=bias_sb, in_=bias_d[0:1, :])
                ones_sb = consts.tile([1, P], f16)
                nc.scalar.dma_start(out=ones_sb, in_=bias_d[1:2, 0:P])
            # gpsimd SWDGE (q0): late-needed tiles
            leaf_sb = consts.tile([P, 2, O], f16)
            nc.gpsimd.dma_start(
                out=leaf_sb, in_=leaf_d[:, :].rearrange("p (c o) -> p c o", c=2)
            )
            nc.gpsimd.dma_start(out=xts[3], in_=xsrc(3))
            nc.gpsimd.dma_start(out=xts[7], in_=xsrc(7))
            ident = consts.tile([P, P], f16)
            masks.make_identity(nc, ident[:, :])
            if apply_affine:
                gamma_sb = consts.tile([P, O], f32)
                nc.gpsimd.dma_start(
                    out=gamma_sb, in_=gamma_d[:, :].to_broadcast((P, O))
                )
                beta_sb = consts.tile([P, O], f32)
                nc.gpsimd.dma_start(out=beta_sb, in_=beta_d[:, :].to_broadcast((P, O)))

            state = {}
            gz = gatep.tile([P, NTILES], f32, name="gz", tag="gz")

            sa_ctx = {}

            def stage_a1(q):
                """mm1 + sigmoid + gate-complement for supertile q."""
                ps1 = psum1.tile([P, 2, COLS], f32, name="ps1")
                for a in range(2):
                    j = 2 * q + a
                    for k in range(KT):
                        nc.tensor.matmul(
                            ps1[:, a], xsel(j, k), wsel(k), start=(k == 0),
                            stop=(not has_bias and k == KT - 1),
                        )
                    if has_bias:
                        nc.tensor.matmul(
                            ps1[:, a], ones_sb, bias_sb, start=False, stop=True
                        )

                # ---- single sigmoid pass over all 256 columns ----
                d2 = d2p.tile([P, 2, COLS], f16, name="d2")
                nc.scalar.activation(out=d2, in_=ps1, func=Act.Sigmoid)
                # gate complement sigma(-g) for e = s/(1-s)
                som = gatep.tile([P, 2, T], f16, name="som")
                nc.scalar.activation(
                    out=som, in_=ps1[:, :, NDEC:COLS], func=Act.Sigmoid, scale=-1.0
                )
                sa_ctx[q] = (d2, som)

            def stage_a2(q):
                """gate e = s/(1-s) + bit-plane cascade for supertile q;
                emitted after mm2_pair so the DVE stats/chain of the older
                pair aren't head-of-line blocked behind these later-ready
                ops."""
                d2, som = sa_ctx.pop(q)
                gr = gatep.tile([P, 2, T], f32, name="gr")
                nc.vector.reciprocal(gr, som)
                ge = gatep.tile([P, 2, T], f16, name="ge")
                nc.vector.scalar_tensor_tensor(
                    out=ge, in0=d2[:, :, NDEC:COLS], scalar=1.0, in1=gr,
                    op0=Alu.mult, op1=Alu.mult,
                )
                nc.vector.reduce_sum(gz[:, 2 * q : 2 * q + 2], ge, axis=X)

                # ---- bit-plane cascade, all unit-stride plain TT ops ----
                # level-(d+1) block cols: [c*T + t], node 2^d-1+bitrev(c).
                # H_d index v_d (LSB-first path prefix), tree-inner.
                # om-halves via H*(s-1) = H*s - H (tensor_sub of the s-half);
                # the sign is folded into leafp2 rows.  H1/H2 on DVE (tiny),
                # H3/H4 on GpSimd.
                roots = d2[:, :, 0:T]
                l2 = d2[:, :, T : 3 * T].rearrange("p a (c t) -> p a c t", c=2)
                l3 = d2[:, :, 3 * T : 7 * T].rearrange("p a (c t) -> p a c t", c=4)
                l4 = d2[:, :, 7 * T : 15 * T].rearrange("p a (c t) -> p a c t", c=8)
                h1 = casc.tile([P, 2, 2, T], f16, name="h1")
                nc.vector.tensor_mul(h1[:, :, 0], ge, roots)
                nc.vector.tensor_sub(h1[:, :, 1], h1[:, :, 0], ge)
                h2 = casc.tile([P, 2, 4, T], f16, name="h2")
                nc.vector.tensor_mul(h2[:, :, 0:2], h1, l2)
                nc.vector.tensor_sub(h2[:, :, 2:4], h2[:, :, 0:2], h1)
                # supertile 3's cascade gates the final transposes+mm2: the
                # gpsimd queue retires it late, so run the last one on DVE
                # where the queue is clear right after H2(3)
                e34 = nc.vector if q == NST - 1 else nc.gpsimd
                h3 = casc.tile([P, 2, 8, T], f16, name="h3")
                e34.tensor_mul(h3[:, :, 0:4], h2, l3)
                e34.tensor_sub(h3[:, :, 4:8], h3[:, :, 0:4], h2)
                acc = accp.tile([P, 2, NL, T], f16, name="acc")
                e34.tensor_mul(acc[:, :, 0:8], h3, l4)
                e34.tensor_sub(acc[:, :, 8:16], acc[:, :, 0:8], h3)
                state[q] = acc

            def ln_finalize(jbase, w, mvw, pair, split_dma=False):
                """LN for tiles jbase..jbase+w-1: rstd via magic seed + one
                Newton step, apply on ACT, out DMA on sync/scalar."""
                fin_prio = tc.high_priority(offset=120)
                fin_prio.__enter__()
                gzb = gz[:, jbase : jbase + w]
                # vt = var + eps*Z^2
                vt = smalls.tile([P, 2], f32, name="vt")[:, 0:w]
                nc.vector.scalar_tensor_tensor(
                    out=vt, in0=gzb, scalar=float(EPS), in1=gzb,
                    op0=Alu.mult, op1=Alu.mult,
                )
                nc.vector.tensor_add(vt, vt, mvw[:, :, 1])
                iv = smalls.tile([P, 2], i32, name="iv")[:, 0:w]
                nc.vector.tensor_scalar(
                    out=iv, in0=vt.bitcast(i32), scalar1=1, scalar2=None,
                    op0=Alu.logical_shift_right,
                )
                yt = smalls.tile([P, 2], f32, name="yt")[:, 0:w]
                nc.vector.tensor_scalar(
                    out=yt.bitcast(i32), in0=iv, scalar1=-1, scalar2=MAGIC,
                    op0=Alu.mult, op1=Alu.add,
                )
                aq = smalls.tile([P, 2], f32, name="aq")[:, 0:w]
                nc.vector.tensor_mul(aq, yt, yt)
                bq = smalls.tile([P, 2], f32, name="bq")[:, 0:w]
                nc.vector.scalar_tensor_tensor(
                    out=bq, in0=vt, scalar=0.5, in1=aq, op0=Alu.mult, op1=Alu.mult
                )
                cq = smalls.tile([P, 2], f32, name="cq")[:, 0:w]
                nc.vector.tensor_scalar(
                    out=cq, in0=bq, scalar1=-1.0, scalar2=1.5,
                    op0=Alu.mult, op1=Alu.add,
                )
                nc.vector.tensor_mul(yt, yt, cq)
                nb = smalls.tile([P, 2], f32, name="nb")[:, 0:w]
                nc.vector.scalar_tensor_tensor(
                    out=nb, in0=mvw[:, :, 0], scalar=-1.0, in1=yt,
                    op0=Alu.mult, op1=Alu.mult,
                )
                for idx, (j, ps3) in enumerate(pair):
                    out_sb = outp.tile([P, O], f16, name="out_sb")
                    nc.scalar.activation(
                        out=out_sb, in_=ps3, func=Act.Identity,
                        bias=nb[:, idx : idx + 1], scale=yt[:, idx : idx + 1],
                    )
                    if apply_affine:
                        nc.vector.tensor_mul(out_sb, out_sb, gamma_sb)
                        nc.vector.tensor_add(out_sb, out_sb, beta_sb)
                    dst = out_d[j * P : (j + 1) * P, :]
                    if split_dma and j == NTILES - 1:
                        half = O // 2
                        nc.scalar.dma_start(out=dst[:, 0:half], in_=out_sb[:, 0:half])
                        nc.sync.dma_start(out=dst[:, half:O], in_=out_sb[:, half:O])
                    else:
                        eng = nc.scalar if j >= 6 else nc.sync
                        eng.dma_start(out=dst, in_=out_sb)
                fin_prio.__exit__(None, None, None)

            wts = {}

            def transposes(p):
                """PE transposes of acc(p) + the ACT psum->sbuf copy; the
                copy overlaps the mm1 block emitted right after."""
                acc = state.pop(p)
                accf = acc.rearrange("p a v t -> p (a v t)")
                psT = psumT.tile([P, 4, P], f16, name="psT")
                for c in range(4):
                    nc.tensor.transpose(psT[:, c], accf[:, c * P : (c + 1) * P], ident)
                wt = wtp.tile([P, 4, P], f16, name="wt")
                nc.scalar.copy(
                    wt.rearrange("p c t -> p (c t)"), psT.rearrange("p c t -> p (c t)")
                )
                wts[p] = wt

            def mm2_pair(p):
                wt = wts.pop(p)
                last = p == NST - 1
                mvp = gatep.tile([P, 2, 2], f32, name="mvp")
                pair = []
                # last pair: tile 7 first and finalized per-tile, so its out
                # DMA (split across both queues) starts as early as possible
                for a in (1, 0) if last else (0, 1):
                    j = 2 * p + a
                    ps3 = psum3.tile([P, O], f32, name="ps3")
                    nc.tensor.matmul(
                        ps3, wt[:, 2 * a], leaf_sb[:, 0], start=True, stop=False
                    )
                    nc.tensor.matmul(
                        ps3, wt[:, 2 * a + 1], leaf_sb[:, 1], start=False, stop=True
                    )
                    st6 = smalls.tile([P, 6], f32, name="st6")
                    nc.vector.bn_stats(st6, ps3)
                    nc.vector.bn_aggr(mvp[:, a], st6)
                    if last:
                        ln_finalize(
                            j, 1, mvp[:, a : a + 1], [(j, ps3)], split_dma=True
                        )
                    else:
                        pair.append((j, ps3))
                if not last:
                    ln_finalize(2 * p, 2, mvp, pair)

            # ---- software pipeline, three stages deep: transposes(p) go
            # before mm1(q) so the psum->sbuf copy hides under the mm1
            # block, and mm2(p) lands right after it; the gpsimd cascade of
            # supertile p gets two whole mm1 blocks of slack ----
            for it in range(NST + 3):
                q, p = it, it - 3
                if p >= 0:
                    transposes(p)
                if q < NST:
                    stage_a1(q)
                if p >= 0:
                    mm2_pair(p)
                if q < NST:
                    stage_a2(q)

    return nc


def _host_prep(x, decision_weights, decision_biases, leaf_outputs, gate_w, gate_b,
               node_temp_logits, ln_gamma, ln_beta):
    """Fold temperatures into weights/biases, permute columns for the
    bit-plane cascade, sign/permute leaf rows, cast fp16, shard tokens."""
    x = np.asarray(x, np.float32)
    temps = np.log1p(np.exp(np.asarray(node_temp_logits, np.float64) + 0.5413))
    temps = temps.astype(np.float32)  # TEMP == 1.0
    wd = (np.asarray(decision_weights, np.float32) / temps[..., None]).reshape(NDEC, D)
    order = _dec_order()
    wcat = np.concatenate(
        [wd[order], np.asarray(gate_w, np.float32).T], axis=0
    )  # [256, D]
    # wcatp[p, k*COLS + c] = wcat[c, k*P + p]
    wcatp = np.ascontiguousarray(
        wcat.T.reshape(KT, P, COLS).transpose(1, 0, 2).reshape(P, KT * COLS)
    ).astype(np.float16)
    biasrow = np.concatenate(
        [
            (np.asarray(decision_biases, np.float32) / temps).reshape(NDEC)[order],
            np.asarray(gate_b, np.float32),
        ]
    )
    biascat = np.stack([biasrow, np.ones(COLS, np.float32)]).astype(np.float16)
    # leafp2[p, c*O + o] = sign[c*P+p] * leaf_outputs[perm[c*P+p], o]
    perm, sign = _leaf_perm_signs()
    L2 = sign[:, None] * np.asarray(leaf_outputs, np.float32).reshape(T * NL, O)[perm]
    leafp2 = np.ascontiguousarray(
        L2.reshape(2, P, O).transpose(1, 0, 2).reshape(P, 2 * O)
    ).astype(np.float16)
    # xp[j*P + p, k*P + t] = xT[k*P + p, j*P + t] per core
    tokens = x.reshape(NCORES, TOK_PC, D)
    xps = []
    for c in range(NCORES):
        xT = tokens[c].T.astype(np.float16)  # [D, TOK_PC]
        xp = (
            xT.reshape(KT, P, NTILES, P)
            .transpose(2, 1, 0, 3)
            .reshape(NTILES * P, KT * P)
        )
        xps.append(np.ascontiguousarray(xp))
    gamma = np.asarray(ln_gamma, np.float32)
    beta = np.asarray(ln_beta, np.float32)
    affine = not (np.all(gamma == 1.0) and np.all(beta == 0.0))
    has_bias = bool(np.any(biasrow != 0.0))
    # merged head: wcat k0:2 cols | xt0 k0:2 per core
    heads = [
        np.ascontiguousarray(
            np.concatenate([wcatp[:, 0 : 2 * COLS], xp[0:P, 0 : 2 * P]], axis=1)
        )
        for xp in xps
    ]
    return xps, wcatp, biascat, leafp2, gamma, beta, affine, has_bias, heads


_BUILT = {}


def _get_module(apply_affine, has_bias):
    key = (apply_affine, has_bias)
    if key not in _BUILT:
        nc = build(apply_affine, has_bias)
        nc.compile()
        _BUILT[key] = nc
    return _BUILT[key]


def run_shards(in_maps, apply_affine=False, has_bias=False, trace=False):
    from concourse.bass_utils import run_bass_kernel_spmd

    nc = _get_module(apply_affine, has_bias)
    return run_bass_kernel_spmd(nc, in_maps, list(range(NCORES)), trace=trace)


def make_in_maps(inputs):
    (xps, wcatp, biascat, leafp2, gamma, beta, affine, has_bias, heads) = _host_prep(
        **inputs
    )
    in_maps = []
    for c in range(NCORES):
        m = {"xp": xps[c], "wcatp": wcatp, "leafp2": leafp2, "headp": heads[c]}
        if has_bias:
            m["biascat"] = biascat
        if affine:
            m["gamma"] = gamma[None, :]
            m["beta"] = beta[None, :]
        in_maps.append(m)
    return in_maps, affine, has_bias


def kernel(**inputs) -> np.ndarray:
    B, S = inputs["x"].shape[:2]
    in_maps, affine, has_bias = make_in_maps(inputs)
    res = run_shards(in_maps, apply_affine=affine, has_bias=has_bias)
    out = np.concatenate([res.results[c]["out"] for c in range(NCORES)], axis=0)
    return out.reshape(B, S, O).astype(np.float32)
